# revision 66
# baseline (speedup 1.0000x reference)
"""Trainium2 Bass kernel for the fused attention+LN+GELU+projection module.

Shapes (hardcoded): x [B=256, S=512, D=512]; k/q/v_w [H=256, D]; attn_bias [S, H];
out_w [D, S*H]; output [B, 1, D].

Distribution across 8 NeuronCores:
 - attention (QKV proj, scores, softmax, apply, +bias, LN, GELU): data-parallel
   over batch, 32 batches/core, processed in NCHUNK chunks of batches.
 - out-projection: contraction dim S*H sharded 8 ways; per chunk, an AllToAll
   redistributes that chunk's activations (bf16) from batch-sharded to
   contraction-sharded layout, DMA-XBAR transposes build [k, gb] stationary
   tiles, each core multiplies against its 1/8 slice of out_w (bf16), and a
   per-chunk AllReduce sums the partial outputs. The dependency-driven tile
   scheduler overlaps chunk c's collective + projection with chunk c+1's
   attention.
"""

import sys

sys.path.insert(0, "/opt/trn_rl_repo")

import ml_dtypes
import numpy as np

import concourse.bacc as bacc
import concourse.tile as tile
from concourse import mybir
from concourse.bass_utils import run_bass_kernel_spmd
from concourse.hw_specs import get_activation_tables
from concourse.tile_rust import add_dep_helper
import bass_rust as _bass_rust

N_CORES = 8
B, S, H, D = 256, 512, 256, 512
NB = B // N_CORES          # batches per core (32)
SCALE = 1.0 / (B ** 0.5)   # score scale (batch-size based, faithful to ref)
LN_EPS = 1e-5
NDT = D // 128             # 4 d-tiles
NST = S // 128             # 4 s-tiles
NHT = H // 128             # 2 h-tiles
SLICE = (S // N_CORES) * H  # 16384 contraction elems per core
NC_T = SLICE // 128        # 128 contraction tiles per core
NCHUNK = 2                 # batch chunks (a2a/phase8 granularity)
CB = NB // NCHUNK          # batches per chunk (16)
G = 8                      # deferred-GELU half-chunk size
NQ = NC_T // 4             # ow quads per chunk pass (32)

F32 = mybir.dt.float32
F32R = mybir.dt.float32r
BF16 = mybir.dt.bfloat16
AF = mybir.ActivationFunctionType
ALU = mybir.AluOpType


class _Bacc(bacc.Bacc):
    """Bacc whose activation-table binding is restricted so that exp/ln are
    only servable by natural_log_exp_and_others and gelu by gelu_and_others.
    Avoids per-op ACT_TABLE_LOAD thrash (~2.7us each) from the default
    first-match binding. Table ids keep their act_info.json order."""

    def insert_act_table_loads(self):
        has_activation = any(
            isinstance(i, mybir.InstActivation)
            for b in self.main_func.blocks
            for i in b.instructions
        )
        if not has_activation:
            return
        keep = {"natural_log_exp_and_others", "gelu_and_others"}
        strip = {AF.Exp, AF.Ln, AF.Gelu}
        tables = []
        for name, funcs in get_activation_tables(self.m.arch).items():
            if name not in keep:
                funcs = funcs - strip
            tables.append((name, funcs))
        _bass_rust.insert_act_table_loads(self, tables)


def _build(ln_trivial: bool):
    nc = _Bacc("TRN2", target_bir_lowering=False, debug=False,
               num_devices=N_CORES)

    # ---- DRAM I/O ----
    # x per batch: [128 d-part, 4 dt, 512 s] so one DMA loads a whole batch
    xT = nc.dram_tensor("xT", [NB, 128, NDT, S], F32R, kind="ExternalInput").ap()
    kq_wT = nc.dram_tensor("kq_wT", [NDT, 128, 2 * H], F32R, kind="ExternalInput").ap()
    v_wT = nc.dram_tensor("v_wT", [NDT, 128, H], F32R, kind="ExternalInput").ap()
    kq_b = nc.dram_tensor("kq_b", [128, 2 * H], F32, kind="ExternalInput").ap()
    v_b2 = nc.dram_tensor("v_b2", [NHT, 128, 1], F32, kind="ExternalInput").ap()
    ab = nc.dram_tensor("ab", [NST, 128, H], F32, kind="ExternalInput").ap()
    outb8 = nc.dram_tensor("outb8", [128, D], F32, kind="ExternalInput").ap()
    ones128 = nc.dram_tensor("ones128", [128, 128], F32R, kind="ExternalInput").ap()
    eye128 = nc.dram_tensor("eye128", [128, 128], F32R, kind="ExternalInput").ap()
    eye16 = nc.dram_tensor("eye16", [128, 128], BF16, kind="ExternalInput").ap()
    # out_w slice, bf16, quad layout: [q][128 k-part][4 kt][512 d]
    owT = nc.dram_tensor("owT", [NQ, 128, 4, D], BF16, kind="ExternalInput").ap()
    if not ln_trivial:
        lng = nc.dram_tensor("lng", [128, H], F32, kind="ExternalInput").ap()
        lnb = nc.dram_tensor("lnb", [128, H], F32, kind="ExternalInput").ap()
    y_out = nc.dram_tensor("y", [N_CORES * NB, D], F32, kind="ExternalOutput").ap()

    # internal DRAM (collective bounce buffers), per chunk
    a2a_in = [nc.dram_tensor(f"a2a_in{c}", [N_CORES, CB, S // N_CORES, H],
                             BF16).ap() for c in range(NCHUNK)]
    a2a_out = [nc.dram_tensor(f"a2a_out{c}", [N_CORES * CB, SLICE],
                              BF16).ap() for c in range(NCHUNK)]
    yb = [nc.dram_tensor(f"yb{c}", [N_CORES * CB, D], F32).ap()
          for c in range(NCHUNK)]
    yred = [nc.dram_tensor(f"yred{c}", [N_CORES * CB, D], F32).ap()
            for c in range(NCHUNK)]
    import os
    dbg = dbgy = None
    if os.environ.get("KDBG"):
        dbg = nc.dram_tensor("dbg", [N_CORES * CB, SLICE], BF16,
                             kind="ExternalOutput").ap()
        dbgy = nc.dram_tensor("dbgy", [NCHUNK, N_CORES * CB, D], F32,
                              kind="ExternalOutput").ap()

    from contextlib import ExitStack
    with tile.TileContext(nc) as tc, ExitStack() as ctx:
        if True:
            constp = ctx.enter_context(tc.tile_pool(name="const", bufs=1))
            xtp = ctx.enter_context(tc.tile_pool(name="xt", bufs=3))
            kqp = ctx.enter_context(tc.tile_pool(name="kqsb", bufs=8))
            vtp = ctx.enter_context(tc.tile_pool(name="vtsb", bufs=4))
            ep = ctx.enter_context(tc.tile_pool(name="esb", bufs=4))
            wp = ctx.enter_context(tc.tile_pool(name="wsb", bufs=4))
            recp = ctx.enter_context(tc.tile_pool(name="rec", bufs=3))
            tp = ctx.enter_context(tc.tile_pool(name="tsb", bufs=G + 8))
            statp = ctx.enter_context(tc.tile_pool(name="stat", bufs=G + 8))
            actp = ctx.enter_context(tc.tile_pool(name="actsb", bufs=3))
            p8wp = ctx.enter_context(tc.tile_pool(name="p8w", bufs=8))
            p8Lp = ctx.enter_context(tc.tile_pool(name="p8L", bufs=2))
            p8sp = ctx.enter_context(tc.tile_pool(name="p8s", bufs=8))
            ysbp = ctx.enter_context(tc.tile_pool(name="ysb", bufs=2))
            # ---- persistent constants ----
            kqw_sb = []
            vw_sb = []
            for dt_ in range(NDT):
                t = constp.tile([128, 2 * H], F32R, tag=f"kqw{dt_}")
                nc.sync.dma_start(t[:], kq_wT[dt_])
                kqw_sb.append(t)
                t = constp.tile([128, H], F32R, tag=f"vw{dt_}")
                nc.sync.dma_start(t[:], v_wT[dt_])
                vw_sb.append(t)
            kqb_sb = constp.tile([128, 2 * H], F32, tag="kqb")
            nc.sync.dma_start(kqb_sb[:], kq_b[:])
            vb_sb = []
            for ht in range(NHT):
                t = constp.tile([128, 1], F32, tag=f"vb{ht}")
                nc.sync.dma_start(t[:], v_b2[ht])
                vb_sb.append(t)
            ab_sb = []
            for st in range(NST):
                t = constp.tile([128, H], F32, tag=f"ab{st}")
                nc.sync.dma_start(t[:], ab[st])
                ab_sb.append(t)
            outb_sb = constp.tile([128, D], F32, tag="outb")
            nc.sync.dma_start(outb_sb[:], outb8[:])
            if not ln_trivial:
                lng_sb = constp.tile([128, H], F32, tag="lng")
                nc.sync.dma_start(lng_sb[:], lng[:])
                lnb_sb = constp.tile([128, H], F32, tag="lnb")
                nc.sync.dma_start(lnb_sb[:], lnb[:])
            ones_sb = constp.tile([128, 128], F32R, tag="ones")
            nc.sync.dma_start(ones_sb[:], ones128[:])
            ones_col = ones_sb[:, 0:1]
            ones_row = ones_sb[0:1, :]
            eye_sb = constp.tile([128, 128], F32R, tag="eye")
            nc.sync.dma_start(eye_sb[:], eye128[:])
            eye16_sb = constp.tile([128, 128], BF16, tag="eye16")
            nc.sync.dma_start(eye16_sb[:], eye16[:])
            eps_sb = constp.tile([128, 1], F32, tag="eps")
            nc.gpsimd.memset(eps_sb[:], LN_EPS)

            projps = ctx.enter_context(
                tc.tile_pool(name="projps", bufs=2, space="PSUM"))
            scps = ctx.enter_context(
                tc.tile_pool(name="scps", bufs=1, space="PSUM"))
            smps = ctx.enter_context(
                tc.tile_pool(name="smps", bufs=1, space="PSUM"))
            apps = ctx.enter_context(
                tc.tile_pool(name="apps", bufs=1, space="PSUM"))
            yps = ctx.enter_context(
                tc.tile_pool(name="yps", bufs=1, space="PSUM"))
            trpps = ctx.enter_context(
                tc.tile_pool(name="trpps", bufs=2, space="PSUM"))
            if True:
                grp_tbl_insts = []   # current half-chunk exp/ln ACT instrs
                prev_gelu = None     # last gelu instruction of previous burst

                def attention_batch(b, c):
                    nonlocal prev_gelu
                    xt = xtp.tile([128, NDT, S], F32R, tag="xt")
                    nc.sync.dma_start(xt[:], xT[b])

                    # vT[h, s] = sum_d v_wT[d, h] * xT[d, s]  (+v_b per-part)
                    vt_sb = []
                    for ht in range(NHT):
                        ps = projps.tile([128, S], F32, tag="proj")
                        for dt_ in range(NDT):
                            nc.tensor.matmul(
                                ps[:], vw_sb[dt_][:, ht * 128:(ht + 1) * 128],
                                xt[:, dt_, :],
                                start=(dt_ == 0), stop=(dt_ == NDT - 1))
                        t = vtp.tile([128, S], F32R, tag="vt")
                        nc.vector.tensor_scalar(t[:], ps[:], vb_sb[ht][:],
                                                None, ALU.add)
                        vt_sb.append(t)

                    # kq[s, j] = sum_d x[s, d] * [k_wT | q_wT][d, j]  (+bias)
                    kq_sb = []
                    for st in range(NST):
                        ps = projps.tile([128, S], F32, tag="proj")
                        for dt_ in range(NDT):
                            nc.tensor.matmul(
                                ps[:], xt[:, dt_, st * 128:(st + 1) * 128],
                                kqw_sb[dt_][:],
                                start=(dt_ == 0), stop=(dt_ == NDT - 1))
                        t = kqp.tile([128, 2 * H], F32R, tag="kq")
                        nc.vector.tensor_add(t[:], ps[:], kqb_sb[:])
                        kq_sb.append(t)

                    # scores[h, g] = sum_s k[s, h] q[s, g]; e = exp(scores/16)
                    e_sb = []
                    sc = scps.tile([128, NHT, H], F32, tag="sc")
                    for ht in range(NHT):
                        for st in range(NST):
                            nc.tensor.matmul(
                                sc[:, ht, :],
                                kq_sb[st][:, ht * 128:(ht + 1) * 128],
                                kq_sb[st][:, H:2 * H],
                                start=(st == 0), stop=(st == NST - 1))
                        t = ep.tile([128, H], F32R, tag="e")
                        ei = nc.scalar.activation(t[:], sc[:, ht, :], AF.Exp,
                                                  scale=SCALE)
                        grp_tbl_insts.append(ei)
                        e_sb.append(t)

                    # softmax denom: d[g] = sum_h e[h, g], computed as columns
                    # (e-block stationary x ones), recip on DVE, PE-transpose
                    # back to a row, then broadcast across partitions. All
                    # PSUM pieces live in one [128, 258] tile: cols 256:258
                    # hold the column denominators, rows 0:1 x cols 0:256 the
                    # reassembled reciprocal row, cols 0:256 the broadcast.
                    sm = smps.tile([128, H + 2], F32, tag="sm")
                    for gt in range(2):
                        for ht in range(NHT):
                            nc.tensor.matmul(
                                sm[:, H + gt:H + gt + 1],
                                e_sb[ht][:, gt * 128:(gt + 1) * 128].bitcast(F32),
                                ones_col.bitcast(F32),
                                start=(ht == 0), stop=(ht == NHT - 1))
                    rec_col = recp.tile([128, 2], F32, tag="reccol")
                    with nc.allow_low_precision(reason="softmax recip"):
                        nc.vector.reciprocal(rec_col[:], sm[:, H:H + 2])
                    for gt in range(2):
                        nc.tensor.matmul(
                            sm[0:1, gt * 128:(gt + 1) * 128],
                            rec_col[:, gt:gt + 1], eye_sb[:].bitcast(F32),
                            start=True, stop=True)
                    rrow_sb = recp.tile([1, H], F32R, tag="rrowsb")
                    nc.scalar.activation(rrow_sb[:], sm[0:1, 0:H], AF.Copy)
                    bc = sm[:, 0:H]
                    nc.tensor.matmul(bc, ones_row, rrow_sb[:],
                                     start=True, stop=True)
                    w_sb = []
                    for ht in range(NHT):
                        t = wp.tile([128, H], F32R, tag="w")
                        nc.vector.tensor_mul(t[:], e_sb[ht][:], bc[:])
                        w_sb.append(t)

                    # out5[s, g] = sum_h vT[h, s] w[h, g]; +attn_bias; LN stats
                    t_all = tp.tile([128, NST, H], F32, tag="tall")
                    p5 = None
                    for st in range(NST):
                        if st % 2 == 0:
                            p5 = apps.tile([128, 2, H], F32, tag="p5")
                        for ht in range(NHT):
                            nc.tensor.matmul(
                                p5[:, st % 2, :],
                                vt_sb[ht][:, st * 128:(st + 1) * 128],
                                w_sb[ht][:],
                                start=(ht == 0), stop=(ht == NHT - 1))
                        nc.vector.tensor_add(t_all[:, st, :], p5[:, st % 2, :],
                                             ab_sb[st][:])
                    st24 = statp.tile([128, NST, 6], F32, tag="st24")
                    mv = statp.tile([128, NST, 2], F32, tag="mv")
                    for st in range(NST):
                        nc.vector.bn_stats(st24[:, st, :], t_all[:, st, :])
                        nc.vector.bn_aggr(mv[:, st, :], st24[:, st, :])
                    # rstd = (var+eps)^-0.5 = exp(-0.5*ln(var+eps)); computed
                    # on the packed [128, 4, 2] stats (mean lanes produce
                    # garbage that is never read).
                    lnv = statp.tile([128, NST, 2], F32, tag="lnv")
                    li = nc.scalar.activation(lnv[:], mv[:], AF.Ln,
                                              bias=eps_sb[:])
                    grp_tbl_insts.append(li)
                    rstd = statp.tile([128, NST, 2], F32, tag="rstd")
                    ri = nc.scalar.activation(rstd[:], lnv[:], AF.Exp,
                                              scale=-0.5)
                    grp_tbl_insts.append(ri)
                    nb_t = statp.tile([128, NST, 1], F32, tag="nb")
                    nc.vector.tensor_mul(nb_t[:], mv[:, :, 0:1],
                                         rstd[:, :, 1:2])
                    nc.vector.tensor_scalar(nb_t[:], nb_t[:], -1.0, None,
                                            ALU.mult)
                    return t_all, rstd, nb_t

                last_tbl_mem = [None]

                def gelu_burst(pend, c):
                    # deferred GELU pass for a finished half-chunk
                    nonlocal prev_gelu, grp_tbl_insts
                    if prev_gelu is not None:
                        for inst in grp_tbl_insts:
                            add_dep_helper(inst.ins, prev_gelu.ins,
                                           sync=False,
                                           reason="act-table grouping")
                    if grp_tbl_insts:
                        last_tbl_mem[0] = grp_tbl_insts[-1]
                    last_tbl = last_tbl_mem[0]
                    grp_tbl_insts = []
                    for pb, t_all, rstd, nb_t in pend:
                        act_sb = actp.tile([128, NST, H], BF16, tag="act")
                        for st in range(NST):
                            if ln_trivial:
                                gi = nc.scalar.activation(
                                    act_sb[:, st, :], t_all[:, st, :], AF.Gelu,
                                    bias=nb_t[:, st, :],
                                    scale=rstd[:, st, 1:2])
                            else:
                                nrm = tp.tile([128, H], F32, tag="nrm")
                                nc.scalar.activation(
                                    nrm[:], t_all[:, st, :], AF.Identity,
                                    bias=nb_t[:, st, 0:1],
                                    scale=rstd[:, st, 1:2])
                                nc.vector.tensor_mul(nrm[:], nrm[:],
                                                     lng_sb[:])
                                nc.vector.tensor_add(nrm[:], nrm[:],
                                                     lnb_sb[:])
                                gi = nc.scalar.activation(
                                    act_sb[:, st, :], nrm[:], AF.Gelu)
                            add_dep_helper(gi.ins, last_tbl.ins,
                                           sync=False,
                                           reason="act-table grouping")
                            # store [128 s, 256 h] as the two 64-row halves
                            # for dest cores 2st / 2st+1 of this chunk's a2a
                            seng = nc.sync if (pb % 2 == 0) else nc.gpsimd
                            seng.dma_start(
                                a2a_in[c][2 * st:2 * st + 2, pb % CB],
                                act_sb[:, st, :])
                            prev_gelu = gi

                def phase8_chunk(c, cc):
                    # y_part[gb, d] = sum_k actT[k, gb] * ow[k, d] over this
                    # core's 16384-k slice, for this chunk's 128 global batches
                    ypsum = yps.tile([128, D], F32, tag="y")
                    # plain DMA loads of the received activations (these wait
                    # correctly on the collective), then PE-transpose 128x128
                    # blocks into [k, gb] stationary tiles.
                    CW = 2048  # k columns per L load
                    for q8 in range(SLICE // CW):
                        L = p8Lp.tile([128, CW], BF16, tag="L")
                        li = nc.sync.dma_start(
                            L[:], a2a_out[c][0:128, q8 * CW:(q8 + 1) * CW])
                        # sliced DRAM reads can miss the RAW edge against the
                        # collective's opt'd-AP output write; pin it explicitly
                        add_dep_helper(li.ins, cc.ins, sync=True,
                                       reason="L load after a2a")
                        owq = None
                        trp = None
                        stat4 = None
                        for i in range(CW // 128):
                            kt = q8 * (CW // 128) + i
                            if kt % 4 == 0:
                                owq = p8wp.tile([128, 4, D], BF16, tag="owq")
                                nc.sync.dma_start(owq[:], owT[kt // 4])
                            if kt % 8 == 0:
                                trp = trpps.tile([128, 8, 128], BF16,
                                                 tag="trp")
                            nc.tensor.transpose(
                                trp[:, kt % 8, :], L[:, i * 128:(i + 1) * 128],
                                eye16_sb[:])
                            if kt % 4 == 3:
                                # copy 4 transposed blocks PSUM->SBUF at once
                                stat4 = p8sp.tile([128, 4, 128], BF16,
                                                  tag="stat")
                                g0 = (kt % 8) - 3
                                if (kt // 4) % 2 == 0:
                                    nc.scalar.activation(
                                        stat4[:], trp[:, g0:g0 + 4, :],
                                        AF.Copy)
                                else:
                                    nc.vector.tensor_copy(
                                        stat4[:], trp[:, g0:g0 + 4, :])
                                for m in range(4):
                                    ktm = kt - 3 + m
                                    nc.tensor.matmul(
                                        ypsum[:], stat4[:, m, :],
                                        owq[:, m, :],
                                        start=(ktm == 0),
                                        stop=(ktm == NC_T - 1))
                    y_sb = ysbp.tile([128, D], F32, tag="ysb")
                    nc.vector.tensor_add(y_sb[:], ypsum[:], outb_sb[:])
                    nc.sync.dma_start(yb[c][:], y_sb[:])
                    if dbgy is not None:
                        nc.sync.dma_start(dbgy[c], y_sb[:])
                    ar = nc.gpsimd.collective_compute(
                        "AllReduce", ALU.add,
                        replica_groups=[list(range(N_CORES))],
                        ins=[yb[c].opt()], outs=[yred[c].opt()])
                    # yred rows r = src*CB + j -> global batch src*NB + c*CB + j
                    yv = y_out.rearrange("(s c j) d -> s c j d",
                                         s=N_CORES, c=NCHUNK)
                    yc = nc.sync.dma_start(
                        yv[:, c],
                        yred[c].rearrange("(s j) d -> s j d", s=N_CORES))
                    add_dep_helper(yc.ins, ar.ins, sync=True,
                                   reason="y copy after allreduce")

                def emit_chunk_tail(c):
                    cc = nc.gpsimd.collective_compute(
                        "AllToAll", ALU.bypass,
                        replica_groups=[list(range(N_CORES))],
                        ins=[a2a_in[c].opt()], outs=[a2a_out[c].opt()])
                    if dbg is not None and c == 0:
                        nc.sync.dma_start(dbg[:], a2a_out[0][:])
                    phase8_chunk(c, cc)

                LAG = 6  # batches of PE runway emitted before a gelu burst
                # group sizes; the last chunk ends with two half-bursts so the
                # final a2a is gated by a shorter serial gelu run
                bounds = [8, 16, 24, 32]
                pend = []
                ready = None          # (pend, chunk, chunk_done)
                due = -1
                for b in range(NB):
                    c = b // CB
                    if ready is not None and b == due:
                        rp, rc, rdone = ready
                        gelu_burst(rp, rc)
                        if rdone:
                            emit_chunk_tail(rc)
                        ready = None
                    pend.append((b,) + attention_batch(b, c))
                    if (b + 1) in bounds:
                        if ready is not None:
                            rp, rc, rdone = ready
                            gelu_burst(rp, rc)
                            if rdone:
                                emit_chunk_tail(rc)
                        cdone = (b + 1) % CB == 0
                        ready = (pend, c, cdone)
                        due = b + 1 + LAG
                        pend = []
                rp, rc, rdone = ready
                gelu_burst(rp, rc)
                emit_chunk_tail(rc)

    nc.compile()
    return nc


_CACHE = {}


def _get_program(ln_trivial):
    if ln_trivial not in _CACHE:
        _CACHE[ln_trivial] = _build(ln_trivial)
    return _CACHE[ln_trivial]


def _prep_inputs(x, k_w, k_b, q_w, q_b, v_w, v_b, attn_bias, ln_g, ln_b,
                 out_w, out_b):
    ln_trivial = bool(np.all(ln_g == 1.0) and np.all(ln_b == 0.0))
    kq_wT = np.ascontiguousarray(
        np.concatenate([k_w.T, q_w.T], axis=1)).reshape(NDT, 128, 2 * H)
    v_wT = np.ascontiguousarray(v_w.T).reshape(NDT, 128, H)
    kq_b = np.ascontiguousarray(
        np.tile(np.concatenate([k_b, q_b])[None, :], (128, 1)))
    v_b2 = np.ascontiguousarray(v_b.reshape(NHT, 128, 1))
    ab = np.ascontiguousarray(attn_bias.reshape(NST, 128, H))
    outb8 = np.ascontiguousarray(np.tile((out_b / N_CORES)[None, :], (128, 1)))
    owT_full = np.ascontiguousarray(out_w.T)  # [S*H, D]
    shared = dict(kq_wT=kq_wT, v_wT=v_wT, kq_b=kq_b, v_b2=v_b2, ab=ab,
                  outb8=outb8, ones128=np.ones((128, 128), np.float32),
                  eye128=np.eye(128, dtype=np.float32),
                  eye16=np.eye(128, dtype=np.float32).astype(
                      ml_dtypes.bfloat16))
    if not ln_trivial:
        shared["lng"] = np.ascontiguousarray(np.tile(ln_g[None, :], (128, 1)))
        shared["lnb"] = np.ascontiguousarray(np.tile(ln_b[None, :], (128, 1)))
    in_maps = []
    for i in range(N_CORES):
        # x: [NB, 512 s, 512 d] -> [NB, 128 d-part, 4 dt, 512 s]
        xi = x[i * NB:(i + 1) * NB].transpose(0, 2, 1).reshape(
            NB, NDT, 128, S).transpose(0, 2, 1, 3)
        xi = np.ascontiguousarray(xi)
        # ow slice: [16384 k, 512 d] -> quads [32 q, 128 k-part, 4 kt, 512 d]
        owi = owT_full[i * SLICE:(i + 1) * SLICE].reshape(
            NQ, 4, 128, D).transpose(0, 2, 1, 3)
        owi = np.ascontiguousarray(owi).astype(ml_dtypes.bfloat16)
        m = dict(shared)
        m["xT"] = xi
        m["owT"] = owi
        in_maps.append(m)
    return ln_trivial, in_maps


def kernel(**inputs):
    xs = {k: np.asarray(v, dtype=np.float32) for k, v in inputs.items()}
    ln_trivial, in_maps = _prep_inputs(
        xs["x"], xs["k_w"], xs["k_b"], xs["q_w"], xs["q_b"], xs["v_w"],
        xs["v_b"], xs["attn_bias"], xs["ln_g"], xs["ln_b"], xs["out_w"],
        xs["out_b"])
    nc = _get_program(ln_trivial)
    res = run_bass_kernel_spmd(nc, in_maps, core_ids=list(range(N_CORES)))
    y = res.results[0]["y"]  # post-AllReduce: identical on every core
    return y.reshape(B, 1, D).astype(np.float32)
